# revision 37
# baseline (speedup 1.0000x reference)
"""Trainium2 Bass kernel for a transformer decoder layer (self-attn + cross-attn + FFN).

Sharding: 8 cores; cores 0-3 handle batch 0, cores 4-7 batch 1; each core owns a
contiguous 512-token slice of queries for every row-wise op.  K/V are computed
sharded (each core projects a 4-head slice) and AllGathered per batch group as
four sub-collectives (K oc0, K oc1, V kc0-7, V kc8-15) so QK can start on the
K quarters before V lands; every ag_out read carries an explicit dependency on
its sub-collective.

Layout: everything on-chip is feature-major (x^T: [d on partitions, t on free]).
Performance structure:
- K bias dropped (softmax is shift-invariant in it - exact), V bias folded into
  the O-projection bias on host (sum softmax = 1), Q/fc1 biases applied in the
  ACT drain, O/fc2 bias+residual fused into one DVE affine_then_add.
- Attention is software-pipelined one pr (d-chunk = 2 heads) deep: AV+denom
  matmuls of the previous pr interleave with QK+exp of the current pr.  QK
  pairs are row-tiled (2 heads on disjoint 64-row PE groups, concurrent); the
  psum ring is one 4-bank [128,2,2,512] plus one 2-bank [128,1,2,512] tile,
  each holding both heads so one exp ACTIVATE covers 2 heads x {2,1} key
  chunks; AV uses col-tiled M=64 pairs (concurrent) plus col-tiled M=1
  denominator-row matmuls riding the same accumulation; softmax division is
  reciprocal_approx_fast on the two denominator rows (single op, partitions
  0..32) + K=1 matmul broadcast planes + one DVE multiply.
- LayerNorm: Sum(x)/Sum(x^2) matmuls accumulate inside the O-proj drain
  callbacks; rstd = Exp(-0.5*Ln(var+eps)) keeps everything in the
  natural_log_exp ACT table set (zero table reloads in the whole kernel);
  per-token scale/shift applied via rank-1 (gamma x row) plane matmuls and
  2 DVE ops per feature chunk.
- dec/enc stream through SBUF in 512-token chunks (never fully resident);
  PSUM is statically partitioned into exactly 4 rings (4+2+1+1 banks) that
  every phase reuses; the final LN streams per-chunk output DMAs.
"""

import sys
import time

for _p in ("/opt/trn_rl_repo", "/root/.axon_site/_ro/trn_rl_repo"):
    if _p not in sys.path:
        sys.path.append(_p)

import numpy as np
import ml_dtypes

B, T, D, H, DH, FFN = 2, 2048, 1024, 16, 64, 4096
N_CORES = 8
CPB = N_CORES // B          # cores per batch
TL = T // CPB               # local tokens per core
DC = D // 128               # d-chunks (8)
KC = T // 128               # key chunks (16)
OC1 = FFN // 128            # fc1 out chunks (32)
NT = T // 512               # 512-wide column tiles over T
EPS = 1e-12
SCALE = 1.0 / 8.0           # 1/sqrt(DH)
MASK_NEG = -80000.0         # additive mask value (pre-scale)
WSCALE = 64.0               # fp8 FFN weight pre-scale

bf16 = ml_dtypes.bfloat16
f8np = ml_dtypes.float8_e4m3   # matches mybir.dt.float8e4

_CACHE = {}
DEBUG_TAPS = False
_PHASE_MARKS = []

KSLICE = 2 * 128            # per-core K/V head-dim slice (4 heads = 256 o-dims)
KPART = KSLICE * T          # gathered K region elems per rank
VROW = 4 * (DH + 1)         # V row: 4 head-halves x (64 dims + ones col)
VPART = T * VROW            # gathered V region elems per rank (incl. ones)
AGLEN = KPART + VPART

# key-chunk schedule per pr: ring of (tile 'A' 2kc | tile 'B' 1kc)
KC_SCHED = [("A", 0, 2), ("B", 2, 1), ("A", 3, 2), ("B", 5, 1), ("A", 6, 2),
            ("B", 8, 1), ("A", 9, 2), ("B", 11, 1), ("A", 12, 2), ("B", 14, 1),
            ("B", 15, 1)]
assert sum(n for _, _, n in KC_SCHED) == KC



def _emit(ctx, tc, nc, aps, use_mask):
    import concourse.bass as bass
    from concourse import mybir
    from contextlib import ExitStack
    dt = mybir.dt
    AF = mybir.ActivationFunctionType
    Alu = mybir.AluOpType
    fp32 = dt.float32
    f32r = dt.float32r
    bf = dt.bfloat16
    f8 = dt.float8e4          # e4m3: K/V gathered in fp8 to halve cc wire time

    consts = ctx.enter_context(tc.tile_pool(name="consts", bufs=1))
    sbA = ctx.enter_context(tc.tile_pool(name="sbA", bufs=1))
    sbX = ctx.enter_context(tc.tile_pool(name="sbX", bufs=2))  # x rings
    smallp = ctx.enter_context(tc.tile_pool(name="smallp", bufs=1))
    small2 = ctx.enter_context(tc.tile_pool(name="small2", bufs=2))
    recp = ctx.enter_context(tc.tile_pool(name="recp", bufs=1))
    # PSUM ring discipline: exactly four (pool, tag, shape) rings = 8 banks.
    psA = ctx.enter_context(tc.tile_pool(name="psA", bufs=1, space="PSUM"))
    psB = ctx.enter_context(tc.tile_pool(name="psB", bufs=1, space="PSUM"))
    ps_av = ctx.enter_context(tc.tile_pool(name="ps_av", bufs=1, space="PSUM"))
    ps_ds = ctx.enter_context(tc.tile_pool(name="ps_ds", bufs=1, space="PSUM"))

    tap_attn_ref = [None]

    def qkA_tile():
        return psA.tile([128, 2, 2, 512], fp32, tag="qkA", name="qkA")

    def qkB_tile():
        return psB.tile([128, 1, 2, 512], fp32, tag="qkB", name="qkB")

    def lin_psum(i):
        t = qkA_tile() if i % 2 == 0 else qkB_tile()
        return t[:, 0, 0, :]

    # ---- constants ----
    ones_bfc = consts.tile([128, 1], bf)
    nc.gpsimd.memset(ones_bfc[:], 1.0)
    ones_f32 = consts.tile([128, 64], fp32)
    nc.gpsimd.memset(ones_f32[:], 1.0)
    ones_f32c = consts.tile([128, 1], fp32)
    nc.gpsimd.memset(ones_f32c[:], 1.0)
    eps_t = consts.tile([1, 1], fp32)
    nc.gpsimd.memset(eps_t[:], EPS)

    bcols = consts.tile([128, 72], fp32, tag="bcols")
    nc.sync.dma_start(bcols[:], aps["bias_cols"][:])
    BC = {"saq": 0, "sao": 8, "caq": 16, "cao": 24, "b2": 32, "b1": 40}

    def bcol(name, oc):
        return bcols[:, BC[name] + oc:BC[name] + oc + 1]

    gbe_t = consts.tile([128, 48], fp32, tag="gbe")
    nc.sync.dma_start(gbe_t[:], aps["gbe_all"][:])

    def be_col(ln_i, dcc):
        return gbe_t[:, (2 * ln_i + 1) * DC + dcc:(2 * ln_i + 1) * DC + dcc + 1]

    def g_col(ln_i, dcc):
        return gbe_t[:, 2 * ln_i * DC + dcc:2 * ln_i * DC + dcc + 1]

    ones_row = consts.tile([1, 128], bf, tag="ones_row")
    nc.gpsimd.memset(ones_row[:], 1.0)

    # ---- persistent activations ----
    xloc_b = sbX.tile([128, DC, TL], bf, tag="xb")
    nc.sync.dma_start(xloc_b[:], aps["xlocT_bf"].rearrange("(c p) t -> p c t", p=128))

    mask_sa = mask_ca = None
    if use_mask:
        mask_sa = sbA.tile([128, KC, TL], bf, tag="mask_sa")
        nc.sync.dma_start(mask_sa[:],
                          aps["amask_saT"].rearrange("(c p) t -> p c t", p=128))
        mask_ca = sbA.tile([128, KC, TL], bf, tag="mask_ca")
        nc.sync.dma_start(mask_ca[:],
                          aps["amask_caT"].rearrange("(c p) t -> p c t", p=128))

    def linear(w_sb, x_sb, out_cb, n=TL, nin=DC):
        for oc in range(DC):
            p = lin_psum(oc)
            for dcc in range(nin):
                nc.tensor.matmul(
                    p[:, 0:n], w_sb[:, dcc, oc * 128:(oc + 1) * 128],
                    x_sb[:, dcc, 0:n], start=(dcc == 0), stop=(dcc == nin - 1))
            out_cb(oc, p[:, 0:n])

    def q_drain(qt, bname):
        def cb(oc, pap):
            nc.scalar.activation(out=qt[:, oc, :], in_=pap, func=AF.Identity,
                                 bias=bcol(bname, oc), scale=1.0)
        return cb

    def o_drain(x_dst, sq_dst, bname, res_sb, stat, scale=1.0):
        """Drain + LN stat accumulation fused: the Sum(x)/Sum(x^2) matmuls
        ride each oc's drain so the LN epilogue starts immediately after the
        last chunk."""
        def cb(oc, pap):
            nc.vector.affine_then_add(
                out=x_dst[:, oc, :], in0=pap, in1=res_sb[:, oc, :],
                scale=scale, bias=bcol(bname, oc))
            nc.scalar.activation(out=sq_dst[:, oc, :], in_=x_dst[:, oc, :],
                                 func=AF.Square, scale=1.0)
            nc.tensor.matmul(stat[0:1, :], ones_f32c[:], x_dst[:, oc, :],
                             start=(oc == 0), stop=(oc == DC - 1))
            nc.tensor.matmul(stat[32:33, :], ones_bfc[:], sq_dst[:, oc, :],
                             start=(oc == 0), stop=(oc == DC - 1),
                             tile_position=(0, 32))
        return cb

    def ln_stat_tile():
        return ps_ds.tile([128, 512], fp32, tag="ds", name="stat")

    # ---------------- layernorm ----------------
    def layernorm(ln_i, x_sb, sq_sb, out_b, stat, out_cb=None,
                  qk_cb=None):
        rows = smallp.tile([1, 4, 512], fp32, tag="rows")
        mrow, msq, vrow, lnv = (rows[:, i, :] for i in range(4))
        nc.scalar.activation(out=mrow, in_=stat[0:1, :], func=AF.Copy,
                             scale=1.0 / D)
        nc.scalar.activation(out=msq, in_=mrow, func=AF.Square, scale=1.0)
        nc.vector.scalar_tensor_tensor(
            out=vrow, in0=stat[32:33, :], scalar=1.0 / D, in1=msq,
            op0=Alu.mult, op1=Alu.subtract)
        nc.scalar.activation(out=lnv, in_=vrow, func=AF.Ln,
                             bias=eps_t[:], scale=1.0)
        rowsb = smallp.tile([1, 2, 512], bf, tag="rowsb")
        rstd, mr = rowsb[:, 0, :], rowsb[:, 1, :]
        nc.scalar.activation(out=rstd, in_=lnv, func=AF.Exp, scale=-0.5)
        nc.vector.tensor_mul(out=mr, in0=mrow, in1=rstd)
        # broadcast planes P1=rstd, P2=m*rstd once per LN; per-dim gamma/beta
        # applied as per-partition columns via tensor_scalar
        pl = qkB_tile()[:, 0]
        nc.tensor.matmul(pl[:, 0, :], ones_row[:], rstd, start=True, stop=True)
        nc.tensor.matmul(pl[:, 1, :], ones_row[:], mr, start=True, stop=True)
        for dcc in range(DC):
            t = small2.tile([128, 2, 512], bf, tag="lnt")
            nc.vector.tensor_mul(out=t[:, 0, :], in0=x_sb[:, dcc, :],
                                 in1=pl[:, 0, :])
            nc.vector.tensor_sub(out=t[:, 1, :], in0=t[:, 0, :],
                                 in1=pl[:, 1, :])
            nc.gpsimd.tensor_scalar(out_b[:, dcc, :], t[:, 1, :],
                                    g_col(ln_i, dcc), be_col(ln_i, dcc),
                                    Alu.mult, Alu.add)
            if out_cb is not None:
                out_cb(dcc)
            if qk_cb is not None:
                qk_cb(dcc)

    # =================== attention-phase pools ===================
    with ExitStack() as attn_ctx:
        wbig = attn_ctx.enter_context(tc.tile_pool(name="wbig", bufs=1))
        wkvp = attn_ctx.enter_context(tc.tile_pool(name="wkvp", bufs=2))
        kthp = attn_ctx.enter_context(
            tc.tile_pool(name="kthp", bufs=1 if DEBUG_TAPS else 2))
        vahp = attn_ctx.enter_context(tc.tile_pool(name="vahp", bufs=3))
        expA_p = attn_ctx.enter_context(tc.tile_pool(name="expA", bufs=5))
        expB_p = attn_ctx.enter_context(tc.tile_pool(name="expB", bufs=6))
        stgp = attn_ctx.enter_context(tc.tile_pool(name="stgp", bufs=2))
        ag_sa_p = attn_ctx.enter_context(
            tc.tile_pool(name="ag_sa", bufs=1, space="DRAM"))
        ag_ca_p = attn_ctx.enter_context(
            tc.tile_pool(name="ag_ca", bufs=1, space="DRAM"))

        def weight_tile(name):
            w = wbig.tile([128, DC, D], bf, tag="w")
            nc.sync.dma_start(w[:], aps[name].rearrange("(c p) o -> p c o", p=128))
            return w

        def kv_and_ag(src_name, wk_name, wv_name, agp):
            """K/V head-slice projection streaming the source in 512-token
            chunks, then FOUR fp8 quarter-collectives in consumption order
            (K oc0, K oc1, V kc0-7, V kc8-15) so QK starts on K oc0 while
            the rest still gathers."""
            wk = wkvp.tile([128, DC, KSLICE], bf, tag="wkv")
            nc.sync.dma_start(wk[:],
                              aps[wk_name].rearrange("(c p) o -> p c o", p=128))
            wv = wkvp.tile([128, DC, KSLICE], bf, tag="wkv")
            nc.sync.dma_start(wv[:],
                              aps[wv_name].rearrange("(c p) o -> p c o", p=128))
            src_ap = aps[src_name].rearrange("(c p) t -> p c t", p=128)
            ag_in = agp.tile([AGLEN], f8, tag="ag_in")
            k_reg = ag_in[0:KPART].rearrange("(oc p t) -> oc p t", oc=2, p=128)
            v_reg = ag_in[KPART:AGLEN].rearrange("(kc p o) -> kc p o",
                                                 kc=KC, p=128)  # o = 4*(DH+1)
            cnt = [0]
            srcs = []
            for nt in range(NT):
                src = srcp.tile([128, DC, 512], f8, tag="src", name="src")
                nc.sync.dma_start(src[:], src_ap[:, :, nt * 512:(nt + 1) * 512])
                srcs.append(src)

            rg = [list(range(CPB)), list(range(CPB, 2 * CPB))]
            outs, ccs = [], []

            def kick(reg_ap, n_elem, tag):
                ag_o = agp.tile([CPB, n_elem], f8, tag=tag, name=tag)
                cc = nc.gpsimd.collective_compute(
                    "AllGather", Alu.bypass, ins=[reg_ap.opt()],
                    outs=[ag_o.opt()], replica_groups=rg)
                outs.append(ag_o)
                ccs.append(cc)

            for oc in range(2):
                for nt in range(NT):
                    cnt[0] += 1
                    p = lin_psum(cnt[0])
                    for dcc in range(DC):
                        nc.tensor.matmul(
                            p, wk[:, dcc, oc * 128:(oc + 1) * 128],
                            srcs[nt][:, dcc, :],
                            start=(dcc == 0), stop=(dcc == DC - 1))
                    stg = stgp.tile([128, 512], f8, tag="stg")
                    nc.vector.tensor_copy(stg[:], p)
                    nc.sync.dma_start(k_reg[oc, :, nt * 512:(nt + 1) * 512],
                                      stg[:])
                kick(ag_in[oc * KPART // 2:(oc + 1) * KPART // 2],
                     KPART // 2, f"ag_k{oc}")

            for nt in range(NT):
                for half in range(2):
                    cnt[0] += 1
                    p = lin_psum(cnt[0]).rearrange("p (a b) -> p a b", a=2)
                    for j in range(2):
                        lc = half * 256 + j * 128
                        for dcc in range(DC):
                            nc.tensor.matmul(
                                p[:, j, :], srcs[nt][:, dcc, lc:lc + 128],
                                wv[:, dcc, :], start=(dcc == 0),
                                stop=(dcc == DC - 1))
                    # stage 65-wide rows: V dims 0-63 + a ones column the AV
                    # matmul uses to accumulate the softmax denominator
                    stg = stgp.tile([128, 2, 4, DH + 1], f8, tag="stgv")
                    nc.vector.tensor_copy(
                        stg[:, :, :, 0:DH],
                        p.rearrange("p a (h d) -> p a h d", h=4))
                    nc.gpsimd.memset(stg[:, :, :, DH:DH + 1], 1.0)
                    for j in range(2):
                        kc = 4 * nt + 2 * half + j
                        nc.sync.dma_start(
                            v_reg[kc], stg[:, j].rearrange("p h d -> p (h d)"))
                if nt == 1:
                    kick(ag_in[KPART:KPART + VPART // 2], VPART // 2, "ag_v0")
            kick(ag_in[KPART + VPART // 2:AGLEN], VPART // 2, "ag_v1")
            return outs, ccs

        # ---------------- attention (pr-pipelined) ----------------
        def attention(qt, ag_pair, mask_sb, attn_out, interleave):
            ag_outs, ag_ccs = ag_pair
            attnU = sbA.tile([128, DC, TL], bf, tag="attnU")
            k_srcs = [ag_outs[i].rearrange("r (p t) -> r p t", p=128)
                      for i in range(2)]
            v_srcs = [ag_outs[2 + i].rearrange(
                "r (kc p hl d) -> r p kc hl d", kc=KC // 2, p=128, hl=4)
                for i in range(2)]

            def fetch(pr):
                from concourse.bass import _add_dep_helper
                ktp_t = kthp.tile([128, T], f8, tag="kth")
                d = nc.sync.dma_start(ktp_t[:], k_srcs[pr % 2][pr // 2])
                _add_dep_helper(d.ins, ag_ccs[pr % 2].ins, sync=True,
                                reason="ktp waits AllGather K quarter")
                # V rows come 65 wide (64 dims + gathered ones column): AV
                # matmul row 64 then accumulates the softmax denominator.
                va = vahp.tile([128, KC, 2, DH + 1], f8, tag="vah")
                hl0 = (2 * pr) % 4
                for q in range(4):  # split across DMA queues + V halves
                    d = nc.sync.dma_start(
                        va[:, 4 * q:4 * q + 4, :, :],
                        v_srcs[q // 2][pr // 2, :, 4 * (q % 2):4 * (q % 2) + 4,
                                       hl0:hl0 + 2, :])
                    _add_dep_helper(d.ins, ag_ccs[2 + q // 2].ins, sync=True,
                                    reason="va waits AllGather V half")
                return ktp_t, va

            def av_emit(pend, kcs):
                pexp, pva, pav0, pav1, _ = pend
                for kc in kcs:
                    e_t, j = pexp[kc]
                    st, sp = kc == 0, kc == KC - 1
                    nc.tensor.matmul(pav0[0:DH + 1, :], pva[:, kc, 0, :],
                                     e_t[:, j, 0, :], start=st, stop=sp)
                    nc.tensor.matmul(pav1[0:DH + 1, :], pva[:, kc, 1, :],
                                     e_t[:, j, 1, :], start=st, stop=sp)

            def av_copyout(pend):
                """Cheap DVE-only drain of the AV psum banks: unnormalized
                attn + denominator rows out to SBUF.  Keeps the next pr's AV
                matmuls from head-of-line-blocking behind the normalization
                chain in the in-order PE queue."""
                _, _, pav0, pav1, ppr = pend
                den_sb = recp.tile([33, 512], fp32, tag="densb")
                nc.vector.tensor_copy(den_sb[0:1, :], pav0[DH:DH + 1, :])
                nc.vector.tensor_copy(den_sb[32:33, :], pav1[DH:DH + 1, :])
                nc.vector.tensor_copy(attnU[0:64, ppr, :], pav0[0:64, :])
                nc.vector.tensor_copy(attnU[64:128, ppr, :], pav1[0:64, :])
                if DEBUG_TAPS and attn_out is tap_attn_ref[0]:
                    nc.sync.dma_start(aps["tap_den"][ppr, 0], den_sb[0:1, :])
                    nc.sync.dma_start(aps["tap_den"][ppr, 1],
                                      den_sb[32:33, :])
                return (den_sb, ppr)

            def av_norm(nstate):
                den_sb, ppr = nstate
                rec = recp.tile([33, 512], fp32, tag="rec")
                nc.vector.reciprocal_approx_fast(out=rec[:, :],
                                                 in_=den_sb[0:33, :])
                plane = qkB_tile()[:, 0, 0, :]
                for hf in range(2):
                    nc.tensor.matmul(
                        plane[64 * hf:64 * hf + 64, :],
                        ones_f32[32 * hf:32 * hf + 1, :],
                        rec[32 * hf:32 * hf + 1, :],
                        start=True, stop=True,
                        tile_position=(32 * hf, 64 * hf))
                plane_sb = small2.tile([128, 512], bf, tag="plane_sb")
                nc.vector.tensor_copy(plane_sb[:], plane[:])
                nc.vector.tensor_mul(out=attn_out[:, ppr, :],
                                     in0=attnU[:, ppr, :],
                                     in1=plane_sb[:])

            pend = None
            norm_pending = None
            pref = fetch(0)
            for it in range(DC + 1):
                if pend is not None:
                    pav0 = ps_av.tile([128, 512], fp32, tag="av")
                    pav1 = ps_ds.tile([128, 512], fp32, tag="ds", name="pav1")
                    pend = (pend[0], pend[1], pav0, pav1, pend[4])
                    av_kc = 0
                if it == DC:
                    if pend is not None:
                        av_emit(pend, range(KC))
                        if norm_pending is not None:
                            av_norm(norm_pending)
                            norm_pending = None
                        av_norm(av_copyout(pend))
                    break
                ktp_t, va = pref
                if it + 1 < DC:
                    pref = fetch(it + 1)

                exp_tiles = {}
                extra = list(interleave[it]) if interleave else []
                ei = 0
                for ci, (kind, kc0, nkc) in enumerate(KC_SCHED):
                    if ci == 2 and norm_pending is not None:
                        av_norm(norm_pending)
                        norm_pending = None
                    if pend is not None:
                        av_emit(pend, range(av_kc, av_kc + nkc))
                        av_kc += nkc
                    ps = qkA_tile() if kind == "A" else qkB_tile()
                    for j in range(nkc):
                        kc = kc0 + j
                        for hf in range(2):
                            po = 64 * hf
                            nc.tensor.matmul(
                                ps[:, j, hf, :],
                                ktp_t[po:po + 64, kc * 128:(kc + 1) * 128],
                                qt[po:po + 64, it, 0:TL],
                                start=True, stop=True, tile_position=(po, 0))
                    if mask_sb is not None:
                        for j in range(nkc):
                            for hf in range(2):
                                nc.vector.tensor_add(
                                    out=ps[:, j, hf, :], in0=ps[:, j, hf, :],
                                    in1=mask_sb[:, kc0 + j, :])
                    pool = expA_p if kind == "A" else expB_p
                    e_t = pool.tile([128, nkc, 2, 512], bf, tag="exp" + kind)
                    nc.scalar.activation(
                        out=e_t[:].rearrange("p a b q -> p (a b q)"),
                        in_=ps[:, 0:nkc].rearrange("p a b q -> p (a b q)"),
                        func=AF.Exp, scale=SCALE)
                    for j in range(nkc):
                        exp_tiles[kc0 + j] = (e_t, j)
                if pend is not None:
                    norm_pending = av_copyout(pend)
                while ei < len(extra):
                    extra[ei]()
                    ei += 1
                pend = (exp_tiles, va, None, None, it)

        # ================= emission =================
        def mark(name):
            _PHASE_MARKS.append((name, nc.next_id()))

        qt_sa = sbA.tile([128, DC, TL], bf, tag="qt")
        attn1 = sbA.tile([128, DC, TL], bf, tag="attn")

        with ExitStack() as src_ctx:
            srcp = src_ctx.enter_context(tc.tile_pool(name="srcp", bufs=8))

            # SA K/V first; each AG quarter kicked as soon as its staging
            # is emitted so the gather overlaps the rest of the prologue.
            mark("sa_kv")
            ag_sa = kv_and_ag("decT_f8", "w_sak", "w_sav", ag_sa_p)
            mark("sa_q")
            wq = weight_tile("w_saq")
            linear(wq, xloc_b, q_drain(qt_sa, "saq"))
            mark("ca_kv")
            ag_ca = kv_and_ag("encT_f8", "w_cak", "w_cav", ag_ca_p)
            mark("sa_attn")
            wo = weight_tile("w_sao")
            tap_attn_ref[0] = attn1
            attention(qt_sa, ag_sa, mask_sa, attn1, None)

        # SA O-proj + bias + residual -> x fp32 (+ squares for LN1)
        mark("sa_o")
        x_sb = sbX.tile([128, DC, TL], fp32, tag="xf")
        sq_sb = sbA.tile([128, DC, TL], bf, tag="sq")
        stat1 = ln_stat_tile()
        linear(wo, attn1, o_drain(x_sb, sq_sb, "sao", xloc_b, stat1))

        if DEBUG_TAPS:
            nc.sync.dma_start(
                aps["tap_attn1"].rearrange("(c p) t -> p c t", p=128), attn1[:])
            nc.sync.dma_start(
                aps["tap_x"].rearrange("(c p) t -> p c t", p=128), x_sb[:])
        mark("ln1")
        wqc = weight_tile("w_caq")
        x1_b = sbX.tile([128, DC, TL], bf, tag="xb")
        qk4 = [None]

        def ln1_qk(dcc):
            # ride CA-Q oc0-3 on the 4 qkA bank slices as x1 chunks appear
            if qk4[0] is None:
                qk4[0] = qkA_tile()
            for oc in range(4):
                nc.tensor.matmul(
                    qk4[0][:, oc // 2, oc % 2, :],
                    wqc[:, dcc, oc * 128:(oc + 1) * 128], x1_b[:, dcc, :],
                    start=(dcc == 0), stop=(dcc == DC - 1))

        qt_ca = sbA.tile([128, DC, TL], bf, tag="qt")
        layernorm(0, x_sb, sq_sb, x1_b, stat1, qk_cb=ln1_qk)
        qdr = q_drain(qt_ca, "caq")
        for oc in range(4):
            qdr(oc, qk4[0][:, oc // 2, oc % 2, :])
        for oc in range(4, DC):
            p = lin_psum(oc)
            for dcc in range(DC):
                nc.tensor.matmul(
                    p, wqc[:, dcc, oc * 128:(oc + 1) * 128], x1_b[:, dcc, :],
                    start=(dcc == 0), stop=(dcc == DC - 1))
            qdr(oc, p)
        if DEBUG_TAPS:
            nc.sync.dma_start(
                aps["tap_x1b"].rearrange("(c p) t -> p c t", p=128), x1_b[:])

        # CA
        mark("ca_q")
        mark("ca_attn")
        attn2 = sbA.tile([128, DC, TL], bf, tag="attn")
        woc = weight_tile("w_cao")
        attention(qt_ca, ag_ca, mask_ca, attn2, None)

        mark("ca_o")
        x2_sb = sbX.tile([128, DC, TL], fp32, tag="xf")
        sq2_sb = sbA.tile([128, DC, TL], bf, tag="sq")
        stat2 = ln_stat_tile()
        linear(woc, attn2, o_drain(x2_sb, sq2_sb, "cao", x1_b, stat2))

    mark("ln2")
    x2_b = sbX.tile([128, DC, TL], bf, tag="xb")
    layernorm(1, x2_sb, sq2_sb, x2_b, stat2)

    # ---------------- FFN ----------------
    with ExitStack() as ffn_ctx:
        mark("ffn1")
        ffnp = ffn_ctx.enter_context(tc.tile_pool(name="ffnp", bufs=1))
        w1p = ffn_ctx.enter_context(tc.tile_pool(name="w1p", bufs=4))
        w2p = ffn_ctx.enter_context(tc.tile_pool(name="w2p", bufs=2))
        ht = ffnp.tile([128, OC1, TL], bf, tag="ht")
        for oc in range(OC1):
            w1 = w1p.tile([128, DC, 128], bf, tag="w1")
            nc.sync.dma_start(
                w1[:],
                aps["w1t"].rearrange("(c p) o -> p c o",
                                     p=128)[:, :, oc * 128:(oc + 1) * 128])
            p = lin_psum(oc)
            for dcc in range(DC):
                nc.tensor.matmul(p, w1[:, dcc, :], x2_b[:, dcc, :],
                                 start=(dcc == 0), stop=(dcc == DC - 1))
            nc.scalar.activation(out=ht[:, oc, :], in_=p, func=AF.Relu,
                                 bias=bcol("b1", oc), scale=1.0)
        mark("ffn2")
        y_sb = sbX.tile([128, DC, TL], fp32, tag="xf")
        sq3_sb = sbA.tile([128, DC, TL], bf, tag="sq")
        stat3 = ln_stat_tile()
        dr3 = o_drain(y_sb, sq3_sb, "b2", x2_b, stat3)
        for oc in range(DC):
            w2 = w2p.tile([128, OC1, 128], bf, tag="w2")
            nc.sync.dma_start(
                w2[:],
                aps["w2t"].rearrange("(c p) o -> p c o",
                                     p=128)[:, :, oc * 128:(oc + 1) * 128])
            p = lin_psum(oc)
            for kc in range(OC1):
                nc.tensor.matmul(p, w2[:, kc, :], ht[:, kc, :],
                                 start=(kc == 0), stop=(kc == OC1 - 1))
            dr3(oc, p)

        mark("ln3")
        out_f = sbX.tile([128, DC, TL], fp32, tag="xf")
        outT = aps["outT"].rearrange("(c p) t -> p c t", p=128)
        layernorm(2, y_sb, sq3_sb, out_f, stat3,
                  out_cb=lambda dcc: nc.sync.dma_start(outT[:, dcc, :],
                                                       out_f[:, dcc, :]))
        mark("end")


def _build(use_mask):
    import concourse.bass as bass
    import concourse.tile as tile
    from concourse import bacc, mybir
    dt = mybir.dt
    nc = bacc.Bacc("TRN2", target_bir_lowering=False, debug=False,
                   num_devices=N_CORES)
    aps = {}

    def inp(name, shape, dtype):
        aps[name] = nc.dram_tensor(name, shape, dtype, kind="ExternalInput").ap()

    inp("decT_f8", [D, T], dt.float8e4)
    inp("encT_f8", [D, T], dt.float8e4)
    inp("xlocT_f32", [D, TL], dt.float32)
    inp("xlocT_bf", [D, TL], dt.bfloat16)
    for nm in ("w_saq", "w_sao", "w_caq", "w_cao"):
        inp(nm, [D, D], dt.bfloat16)
    for nm in ("w_sak", "w_sav", "w_cak", "w_cav"):
        inp(nm, [D, KSLICE], dt.bfloat16)
    inp("w1t", [D, FFN], dt.bfloat16)
    inp("w2t", [FFN, D], dt.bfloat16)
    inp("bias_cols", [128, 72], dt.float32)
    inp("gbe_all", [128, 48], dt.float32)
    if use_mask:
        inp("amask_saT", [T, TL], dt.bfloat16)
        inp("amask_caT", [T, TL], dt.bfloat16)
    aps["outT"] = nc.dram_tensor("outT", [D, TL], dt.float32,
                                 kind="ExternalOutput").ap()
    if DEBUG_TAPS:
        aps["tap_attn1"] = nc.dram_tensor("tap_attn1", [D, TL], dt.bfloat16,
                                          kind="ExternalOutput").ap()
        aps["tap_x"] = nc.dram_tensor("tap_x", [D, TL], dt.float32,
                                      kind="ExternalOutput").ap()
        aps["tap_x1b"] = nc.dram_tensor("tap_x1b", [D, TL], dt.bfloat16,
                                        kind="ExternalOutput").ap()
        aps["tap_den"] = nc.dram_tensor("tap_den", [DC, 2, 512], dt.float32,
                                        kind="ExternalOutput").ap()
        aps["tap_rec"] = nc.dram_tensor("tap_rec", [DC, 2, 512], dt.float32,
                                        kind="ExternalOutput").ap()

    from contextlib import ExitStack
    with tile.TileContext(nc) as tc:
        with ExitStack() as ctx:
            _emit(ctx, tc, nc, aps, use_mask)
    nc.compile()
    return nc


def _make_runner(nc):
    import jax
    from jax.sharding import Mesh, PartitionSpec
    from jax.experimental.shard_map import shard_map
    from concourse import bass2jax, mybir

    bass2jax.install_neuronx_cc_hook()
    part_name = nc.partition_id_tensor.name if nc.partition_id_tensor else None
    in_names, out_names, out_avals = [], [], []
    for alloc in nc.m.functions[0].allocations:
        if not isinstance(alloc, mybir.MemoryLocationSet):
            continue
        name = alloc.memorylocations[0].name
        if alloc.kind == "ExternalInput":
            if name != part_name:
                in_names.append(name)
        elif alloc.kind == "ExternalOutput":
            out_names.append(name)
            out_avals.append(jax.core.ShapedArray(tuple(alloc.tensor_shape),
                                                  mybir.dt.np(alloc.dtype)))
    n_params = len(in_names)
    all_names = list(in_names + out_names)
    if part_name is not None:
        all_names.append(part_name)
    all_names = tuple(all_names)

    def _body(*args):
        operands = list(args)
        if part_name is not None:
            operands.append(bass2jax.partition_id_tensor())
        return tuple(bass2jax._bass_exec_p.bind(
            *operands, out_avals=tuple(out_avals), in_names=all_names,
            out_names=tuple(out_names), lowering_input_output_aliases=(),
            sim_require_finite=True, sim_require_nnan=True, nc=nc))

    devices = jax.devices()[:N_CORES]
    mesh = Mesh(np.asarray(devices), ("core",))
    spec = (PartitionSpec("core"),)
    nin = n_params + len(out_names)
    jfn = jax.jit(
        shard_map(_body, mesh=mesh, in_specs=spec * nin,
                  out_specs=spec * len(out_names), check_rep=False),
        donate_argnums=tuple(range(n_params, nin)), keep_unused=True)

    from jax.sharding import NamedSharding
    shard = NamedSharding(mesh, PartitionSpec("core"))

    def run(in_maps, timeit=False):
        concat_in = [np.concatenate([np.asarray(in_maps[c][n]) for c in range(N_CORES)],
                                    axis=0) for n in in_names]
        zeros = [np.zeros((N_CORES * a.shape[0],) + tuple(a.shape[1:]), a.dtype)
                 for a in out_avals]
        dev_in = [jax.device_put(a, shard) for a in concat_in]
        out = jfn(*dev_in, *[jax.device_put(z, shard) for z in zeros])
        jax.block_until_ready(out)
        times = []
        if timeit:
            # batched back-to-back dispatch; the k-slope of total wall time
            # isolates per-exec device time from fixed batch overhead.
            for _rep in range(2):
                totals = {}
                for k in (4, 44):
                    zsets = [[jax.device_put(z, shard) for z in zeros]
                             for _ in range(k)]
                    jax.block_until_ready(zsets)
                    t0 = time.perf_counter()
                    outs = [jfn(*dev_in, *zs) for zs in zsets]
                    jax.block_until_ready(outs)
                    totals[k] = time.perf_counter() - t0
                times.append((totals[44] - totals[4]) / 40.0)
        per_core = [{n: np.asarray(out[i]).reshape(N_CORES, *out_avals[i].shape)[c]
                     for i, n in enumerate(out_names)} for c in range(N_CORES)]
        return per_core, times

    return run


def _prep_inputs(dec, enc, t_mask, s_mask, weights):
    """Returns (in_maps, use_mask)."""
    use_mask_sa = not bool(np.all(t_mask != 0))
    use_mask_ca = not bool(np.all(s_mask != 0))
    use_mask = use_mask_sa or use_mask_ca

    kvT = weights["_kvT"]
    shared = {k: v for k, v in weights.items() if not k.startswith("_")}
    in_maps = []
    for c in range(N_CORES):
        b, s = c // CPB, c % CPB
        rows = slice(s * TL, (s + 1) * TL)
        osl = slice(s * KSLICE, (s + 1) * KSLICE)
        m = dict(shared)
        for nm in ("w_sak", "w_sav", "w_cak", "w_cav"):
            m[nm] = np.ascontiguousarray(kvT[nm][:, osl])
        decT = np.ascontiguousarray(dec[b].T)
        m["decT_f8"] = decT.astype(f8np)
        m["encT_f8"] = np.ascontiguousarray(enc[b].T).astype(f8np)
        xloc = np.ascontiguousarray(dec[b, rows].T)
        m["xlocT_f32"] = xloc
        m["xlocT_bf"] = xloc.astype(bf16)
        if use_mask:
            am_sa = ((t_mask[0, 0] == 0) * MASK_NEG).astype(np.float32)
            am_ca = ((s_mask[0, 0] == 0) * MASK_NEG).astype(np.float32)
            m["amask_saT"] = np.ascontiguousarray(am_sa[rows].T).astype(bf16)
            m["amask_caT"] = np.ascontiguousarray(am_ca[rows].T).astype(bf16)
        in_maps.append(m)
    return in_maps, use_mask


def _pack_weights(kw):
    """Shared (core-independent) weight tensors."""
    w = {}
    for src, dst in (("sa_wq", "w_saq"), ("sa_wo", "w_sao"), ("ca_wq", "w_caq"),
                     ("ca_wo", "w_cao"), ("w1", "w1t"), ("w2", "w2t")):
        w[dst] = np.ascontiguousarray(np.asarray(kw[src]).T).astype(bf16)

    f32 = np.float32
    # O biases with the V bias folded in: bo' = bo + Wo @ bv
    b_sao = (np.asarray(kw["sa_bo"]) +
             np.asarray(kw["sa_wo"]) @ np.asarray(kw["sa_bv"])).astype(f32)
    b_cao = (np.asarray(kw["ca_bo"]) +
             np.asarray(kw["ca_wo"]) @ np.asarray(kw["ca_bv"])).astype(f32)
    cols = [np.asarray(kw["sa_bq"], f32).reshape(DC, 128).T,
            b_sao.reshape(DC, 128).T,
            np.asarray(kw["ca_bq"], f32).reshape(DC, 128).T,
            b_cao.reshape(DC, 128).T,
            np.asarray(kw["b2"], f32).reshape(DC, 128).T,
            np.asarray(kw["b1"], f32).reshape(OC1, 128).T]
    w["bias_cols"] = np.ascontiguousarray(np.concatenate(cols, axis=1))
    gbe = [np.asarray(kw[k]).astype(f32).reshape(DC, 128).T for k in
           ("g1", "be1", "g2", "be2", "g3", "be3")]
    w["gbe_all"] = np.ascontiguousarray(np.concatenate(gbe, axis=1))
    w["g_rows"] = np.concatenate(
        [np.asarray(kw[k], f32).reshape(1, -1) for k in ("g1", "g2", "g3")],
        axis=1).astype(bf16)
    # K/V weights are head-sharded per core: slices added in _prep_inputs.
    w["_kvT"] = {nm: np.ascontiguousarray(np.asarray(kw[src]).T).astype(bf16)
                 for src, nm in (("sa_wk", "w_sak"), ("sa_wv", "w_sav"),
                                 ("ca_wk", "w_cak"), ("ca_wv", "w_cav"))}
    return w


def _get_runner(use_mask):
    key = bool(use_mask)
    if key not in _CACHE:
        nc = _build(key)
        _CACHE[key] = _make_runner(nc)
    return _CACHE[key]


def kernel(dec, enc, t_mask, s_mask, timeit=False, **kw):
    dec = np.asarray(dec, dtype=np.float32)
    enc = np.asarray(enc, dtype=np.float32)
    weights = _pack_weights(kw)
    in_maps, use_mask = _prep_inputs(dec, enc, np.asarray(t_mask),
                                     np.asarray(s_mask), weights)
    run = _get_runner(use_mask)
    per_core, times = run(in_maps, timeit=timeit)
    out = np.empty((B, T, D), np.float32)
    for c in range(N_CORES):
        b, s = c // CPB, c % CPB
        out[b, s * TL:(s + 1) * TL] = per_core[c]["outT"].T
    if timeit:
        kernel._last_times = times
    return out



# revision 41
# speedup vs baseline: 1.9940x; 1.9940x over previous
"""Trainium2 Bass kernel for a transformer decoder layer (self-attn + cross-attn + FFN).

Sharding: 8 cores; cores 0-3 handle batch 0, cores 4-7 batch 1; each core owns a
contiguous 512-token slice of queries for every row-wise op.  K/V are computed
sharded (each core projects a 4-head slice, fp8 sources) and AllGathered per
batch group as four fp8 quarter-collectives in consumption order (K oc0, K oc1,
V kc0-7, V kc8-15); every gathered read carries an explicit dependency on its
sub-collective.  fp8 (e4m3) for the gathered K/V halves collective wire time;
softmax washes the quantization noise out of the output.

Layout: everything on-chip is feature-major (x^T: [d on partitions, t on free]).
Performance structure:
- K bias dropped (softmax is shift-invariant in it - exact), V bias folded into
  the O-projection bias on host (sum softmax = 1), Q/fc1 biases applied in the
  ACT drain, O/fc2 bias+residual fused into one DVE affine_then_add.
- Attention is software-pipelined one pr (d-chunk = 2 heads) deep, prs ordered
  evens-first so the earliest K quarter feeds the first four prs.  V rows are
  staged 65 wide (64 dims + a gathered ones column) so each AV matmul (M=65)
  accumulates the softmax denominator in psum row 64 for free - no separate
  denominator matmuls.  The psum drain is split: a cheap DVE-only copyout
  (unnormalized attn + den rows) releases the AV banks immediately; the
  normalization (reciprocal_approx_fast + rank-1 reciprocal planes + one DVE
  multiply) is deferred into the next pr so it never head-of-line-blocks the
  in-order PE queue.
- LayerNorm: Sum(x)/Sum(x^2) matmuls ride the O-proj/fc2 drain callbacks;
  rstd = Exp(-0.5*Ln(var+eps)) stays in the natural_log_exp ACT table set
  (zero table reloads); scale/shift = two rank-1 broadcast planes (psum) and
  2 same-engine DVE ops per chunk with (g*P2 - be) precomputed off-path;
  LN1 additionally streams the first half of the CA Q-projection across the
  four qkA psum banks as each x1 chunk is produced.
- dec/enc stream through SBUF in fp8 512-token chunks (never fully resident);
  PSUM is statically partitioned into exactly 4 rings (4+2+1+1 banks) that
  every phase reuses; the final LN streams per-chunk output DMAs.
"""

import sys
import time

for _p in ("/opt/trn_rl_repo", "/root/.axon_site/_ro/trn_rl_repo"):
    if _p not in sys.path:
        sys.path.append(_p)

import numpy as np
import ml_dtypes

B, T, D, H, DH, FFN = 2, 2048, 1024, 16, 64, 4096
N_CORES = 8
CPB = N_CORES // B          # cores per batch
TL = T // CPB               # local tokens per core
DC = D // 128               # d-chunks (8)
KC = T // 128               # key chunks (16)
OC1 = FFN // 128            # fc1 out chunks (32)
NT = T // 512               # 512-wide column tiles over T
EPS = 1e-12
SCALE = 1.0 / 8.0           # 1/sqrt(DH)
MASK_NEG = -80000.0         # additive mask value (pre-scale)
WSCALE = 64.0               # fp8 FFN weight pre-scale

bf16 = ml_dtypes.bfloat16
f8np = ml_dtypes.float8_e4m3   # matches mybir.dt.float8e4

_CACHE = {}
DEBUG_TAPS = False
_PHASE_MARKS = []

KSLICE = 2 * 128            # per-core K/V head-dim slice (4 heads = 256 o-dims)
KPART = KSLICE * T          # gathered K region elems per rank
VROW = 4 * (DH + 1)         # V row: 4 head-halves x (64 dims + ones col)
VPART = T * VROW            # gathered V region elems per rank (incl. ones)
AGLEN = KPART + VPART

# key-chunk schedule per pr: ring of (tile 'A' 2kc | tile 'B' 1kc)
KC_SCHED = [("A", 0, 2), ("B", 2, 1), ("A", 3, 2), ("B", 5, 1), ("A", 6, 2),
            ("B", 8, 1), ("A", 9, 2), ("B", 11, 1), ("A", 12, 2), ("B", 14, 1),
            ("B", 15, 1)]
assert sum(n for _, _, n in KC_SCHED) == KC



def _emit(ctx, tc, nc, aps, use_mask):
    import concourse.bass as bass
    from concourse import mybir
    from contextlib import ExitStack
    dt = mybir.dt
    AF = mybir.ActivationFunctionType
    Alu = mybir.AluOpType
    fp32 = dt.float32
    f32r = dt.float32r
    bf = dt.bfloat16
    f8 = dt.float8e4          # e4m3: K/V gathered in fp8 to halve cc wire time

    consts = ctx.enter_context(tc.tile_pool(name="consts", bufs=1))
    sbA = ctx.enter_context(tc.tile_pool(name="sbA", bufs=1))
    sbX = ctx.enter_context(tc.tile_pool(name="sbX", bufs=2))  # x rings
    smallp = ctx.enter_context(tc.tile_pool(name="smallp", bufs=1))
    small2 = ctx.enter_context(tc.tile_pool(name="small2", bufs=2))
    recp = ctx.enter_context(tc.tile_pool(name="recp", bufs=1))
    # PSUM ring discipline: exactly four (pool, tag, shape) rings = 8 banks.
    psA = ctx.enter_context(tc.tile_pool(name="psA", bufs=1, space="PSUM"))
    psB = ctx.enter_context(tc.tile_pool(name="psB", bufs=1, space="PSUM"))
    ps_av = ctx.enter_context(tc.tile_pool(name="ps_av", bufs=1, space="PSUM"))
    ps_ds = ctx.enter_context(tc.tile_pool(name="ps_ds", bufs=1, space="PSUM"))

    tap_attn_ref = [None]

    def qkA_tile():
        return psA.tile([128, 2, 2, 512], fp32, tag="qkA", name="qkA")

    def qkB_tile():
        return psB.tile([128, 1, 2, 512], fp32, tag="qkB", name="qkB")

    def lin_psum(i):
        t = qkA_tile() if i % 2 == 0 else qkB_tile()
        return t[:, 0, 0, :]

    # ---- constants ----
    ones_bfc = consts.tile([128, 1], bf)
    nc.gpsimd.memset(ones_bfc[:], 1.0)
    ones_f32 = consts.tile([128, 64], fp32)
    nc.gpsimd.memset(ones_f32[:], 1.0)
    ones_f32c = consts.tile([128, 1], fp32)
    nc.gpsimd.memset(ones_f32c[:], 1.0)
    eps_t = consts.tile([1, 1], fp32)
    nc.gpsimd.memset(eps_t[:], EPS)

    bcols = consts.tile([128, 72], fp32, tag="bcols")
    nc.sync.dma_start(bcols[:], aps["bias_cols"][:])
    BC = {"saq": 0, "sao": 8, "caq": 16, "cao": 24, "b2": 32, "b1": 40}

    def bcol(name, oc):
        return bcols[:, BC[name] + oc:BC[name] + oc + 1]

    gbe_t = consts.tile([128, 48], fp32, tag="gbe")
    nc.sync.dma_start(gbe_t[:], aps["gbe_all"][:])

    def be_col(ln_i, dcc):
        return gbe_t[:, (2 * ln_i + 1) * DC + dcc:(2 * ln_i + 1) * DC + dcc + 1]

    def g_col(ln_i, dcc):
        return gbe_t[:, 2 * ln_i * DC + dcc:2 * ln_i * DC + dcc + 1]

    ones_row = consts.tile([1, 128], bf, tag="ones_row")
    nc.gpsimd.memset(ones_row[:], 1.0)

    # ---- persistent activations ----
    xloc_b = sbX.tile([128, DC, TL], bf, tag="xb")
    nc.sync.dma_start(xloc_b[:], aps["xlocT_bf"].rearrange("(c p) t -> p c t", p=128))

    mask_sa = mask_ca = None
    if use_mask:
        mask_sa = sbA.tile([128, KC, TL], bf, tag="mask_sa")
        nc.sync.dma_start(mask_sa[:],
                          aps["amask_saT"].rearrange("(c p) t -> p c t", p=128))
        mask_ca = sbA.tile([128, KC, TL], bf, tag="mask_ca")
        nc.sync.dma_start(mask_ca[:],
                          aps["amask_caT"].rearrange("(c p) t -> p c t", p=128))

    def linear(w_sb, x_sb, out_cb, n=TL, nin=DC):
        for oc in range(DC):
            p = lin_psum(oc)
            for dcc in range(nin):
                nc.tensor.matmul(
                    p[:, 0:n], w_sb[:, dcc, oc * 128:(oc + 1) * 128],
                    x_sb[:, dcc, 0:n], start=(dcc == 0), stop=(dcc == nin - 1))
            out_cb(oc, p[:, 0:n])

    def q_drain(qt, bname):
        def cb(oc, pap):
            nc.scalar.activation(out=qt[:, oc, :], in_=pap, func=AF.Identity,
                                 bias=bcol(bname, oc), scale=1.0)
        return cb

    def o_drain(x_dst, sq_dst, bname, res_sb, stat, scale=1.0):
        """Drain + LN stat accumulation fused: the Sum(x)/Sum(x^2) matmuls
        ride each oc's drain so the LN epilogue starts immediately after the
        last chunk."""
        def cb(oc, pap):
            nc.vector.affine_then_add(
                out=x_dst[:, oc, :], in0=pap, in1=res_sb[:, oc, :],
                scale=scale, bias=bcol(bname, oc))
            nc.scalar.activation(out=sq_dst[:, oc, :], in_=x_dst[:, oc, :],
                                 func=AF.Square, scale=1.0)
            nc.tensor.matmul(stat[0:1, :], ones_f32c[:], x_dst[:, oc, :],
                             start=(oc == 0), stop=(oc == DC - 1))
            nc.tensor.matmul(stat[32:33, :], ones_bfc[:], sq_dst[:, oc, :],
                             start=(oc == 0), stop=(oc == DC - 1),
                             tile_position=(0, 32))
        return cb

    def ln_stat_tile():
        return ps_ds.tile([128, 512], fp32, tag="ds", name="stat")

    # ---------------- layernorm ----------------
    def layernorm(ln_i, x_sb, sq_sb, out_b, stat, out_cb=None,
                  qk_cb=None):
        rows = smallp.tile([1, 4, 512], fp32, tag="rows")
        mrow, msq, vrow, lnv = (rows[:, i, :] for i in range(4))
        nc.scalar.activation(out=mrow, in_=stat[0:1, :], func=AF.Copy,
                             scale=1.0 / D)
        nc.scalar.activation(out=msq, in_=mrow, func=AF.Square, scale=1.0)
        nc.vector.scalar_tensor_tensor(
            out=vrow, in0=stat[32:33, :], scalar=1.0 / D, in1=msq,
            op0=Alu.mult, op1=Alu.subtract)
        nc.scalar.activation(out=lnv, in_=vrow, func=AF.Ln,
                             bias=eps_t[:], scale=1.0)
        rowsb = smallp.tile([1, 2, 512], bf, tag="rowsb")
        rstd, mr = rowsb[:, 0, :], rowsb[:, 1, :]
        nc.scalar.activation(out=rstd, in_=lnv, func=AF.Exp, scale=-0.5)
        nc.vector.tensor_mul(out=mr, in0=mrow, in1=rstd)
        # broadcast planes P1=rstd, P2=m*rstd once per LN; then per chunk:
        # out = (x*P1)*g - (g*P2 - be), with the second factor precomputed
        # off the critical path so each chunk is 2 same-engine DVE ops.
        pl = qkB_tile()[:, 0]
        nc.tensor.matmul(pl[:, 0, :], ones_row[:], rstd, start=True, stop=True)
        nc.tensor.matmul(pl[:, 1, :], ones_row[:], mr, start=True, stop=True)
        p2g = smallp.tile([128, DC, 512], bf, tag="p2g")
        for dcc in range(DC):
            nc.vector.tensor_scalar(p2g[:, dcc, :], pl[:, 1, :],
                                    g_col(ln_i, dcc), be_col(ln_i, dcc),
                                    Alu.mult, Alu.subtract)
        for dcc in range(DC):
            t = small2.tile([128, 512], bf, tag="lnt")
            nc.vector.tensor_mul(out=t[:], in0=x_sb[:, dcc, :],
                                 in1=pl[:, 0, :])
            nc.vector.scalar_tensor_tensor(
                out=out_b[:, dcc, :], in0=t[:], scalar=g_col(ln_i, dcc),
                in1=p2g[:, dcc, :], op0=Alu.mult, op1=Alu.subtract)
            if out_cb is not None:
                out_cb(dcc)
            if qk_cb is not None:
                qk_cb(dcc)

    # =================== attention-phase pools ===================
    with ExitStack() as attn_ctx:
        wbig = attn_ctx.enter_context(tc.tile_pool(name="wbig", bufs=1))
        wkvp = attn_ctx.enter_context(tc.tile_pool(name="wkvp", bufs=2))
        kthp = attn_ctx.enter_context(
            tc.tile_pool(name="kthp", bufs=1 if DEBUG_TAPS else 2))
        vahp = attn_ctx.enter_context(tc.tile_pool(name="vahp", bufs=3))
        expA_p = attn_ctx.enter_context(tc.tile_pool(name="expA", bufs=5))
        expB_p = attn_ctx.enter_context(tc.tile_pool(name="expB", bufs=6))
        stgp = attn_ctx.enter_context(tc.tile_pool(name="stgp", bufs=2))
        ag_sa_p = attn_ctx.enter_context(
            tc.tile_pool(name="ag_sa", bufs=1, space="DRAM"))
        ag_ca_p = attn_ctx.enter_context(
            tc.tile_pool(name="ag_ca", bufs=1, space="DRAM"))

        def weight_tile(name):
            w = wbig.tile([128, DC, D], bf, tag="w")
            nc.sync.dma_start(w[:], aps[name].rearrange("(c p) o -> p c o", p=128))
            return w

        def kv_and_ag(src_name, wk_name, wv_name, agp):
            """K/V head-slice projection streaming the source in 512-token
            chunks, then FOUR fp8 quarter-collectives in consumption order
            (K oc0, K oc1, V kc0-7, V kc8-15) so QK starts on K oc0 while
            the rest still gathers."""
            wk = wkvp.tile([128, DC, KSLICE], bf, tag="wkv")
            nc.sync.dma_start(wk[:],
                              aps[wk_name].rearrange("(c p) o -> p c o", p=128))
            wv = wkvp.tile([128, DC, KSLICE], bf, tag="wkv")
            nc.sync.dma_start(wv[:],
                              aps[wv_name].rearrange("(c p) o -> p c o", p=128))
            src_ap = aps[src_name].rearrange("(c p) t -> p c t", p=128)
            ag_in = agp.tile([AGLEN], f8, tag="ag_in")
            k_reg = ag_in[0:KPART].rearrange("(oc p t) -> oc p t", oc=2, p=128)
            v_reg = ag_in[KPART:AGLEN].rearrange("(kc p o) -> kc p o",
                                                 kc=KC, p=128)  # o = 4*(DH+1)
            cnt = [0]
            srcs = []
            for nt in range(NT):
                src = srcp.tile([128, DC, 512], f8, tag="src", name="src")
                nc.sync.dma_start(src[:], src_ap[:, :, nt * 512:(nt + 1) * 512])
                srcs.append(src)

            rg = [list(range(CPB)), list(range(CPB, 2 * CPB))]
            outs, ccs = [], []

            def kick(reg_ap, n_elem, tag):
                ag_o = agp.tile([CPB, n_elem], f8, tag=tag, name=tag)
                cc = nc.gpsimd.collective_compute(
                    "AllGather", Alu.bypass, ins=[reg_ap.opt()],
                    outs=[ag_o.opt()], replica_groups=rg)
                outs.append(ag_o)
                ccs.append(cc)

            for oc in range(2):
                for nt in range(NT):
                    cnt[0] += 1
                    p = lin_psum(cnt[0])
                    for dcc in range(DC):
                        nc.tensor.matmul(
                            p, wk[:, dcc, oc * 128:(oc + 1) * 128],
                            srcs[nt][:, dcc, :],
                            start=(dcc == 0), stop=(dcc == DC - 1))
                    stg = stgp.tile([128, 512], f8, tag="stg")
                    nc.vector.tensor_copy(stg[:], p)
                    nc.sync.dma_start(k_reg[oc, :, nt * 512:(nt + 1) * 512],
                                      stg[:])
                kick(ag_in[oc * KPART // 2:(oc + 1) * KPART // 2],
                     KPART // 2, f"ag_k{oc}")

            for nt in range(NT):
                for half in range(2):
                    cnt[0] += 1
                    p = lin_psum(cnt[0]).rearrange("p (a b) -> p a b", a=2)
                    for j in range(2):
                        lc = half * 256 + j * 128
                        for dcc in range(DC):
                            nc.tensor.matmul(
                                p[:, j, :], srcs[nt][:, dcc, lc:lc + 128],
                                wv[:, dcc, :], start=(dcc == 0),
                                stop=(dcc == DC - 1))
                    # stage 65-wide rows: V dims 0-63 + a ones column the AV
                    # matmul uses to accumulate the softmax denominator
                    stg = stgp.tile([128, 2, 4, DH + 1], f8, tag="stgv")
                    nc.vector.tensor_copy(
                        stg[:, :, :, 0:DH],
                        p.rearrange("p a (h d) -> p a h d", h=4))
                    nc.gpsimd.memset(stg[:, :, :, DH:DH + 1], 1.0)
                    for j in range(2):
                        kc = 4 * nt + 2 * half + j
                        nc.sync.dma_start(
                            v_reg[kc], stg[:, j].rearrange("p h d -> p (h d)"))
                if nt == 1:
                    kick(ag_in[KPART:KPART + VPART // 2], VPART // 2, "ag_v0")
            kick(ag_in[KPART + VPART // 2:AGLEN], VPART // 2, "ag_v1")
            return outs, ccs

        # ---------------- attention (pr-pipelined) ----------------
        def attention(qt, ag_pair, mask_sb, attn_out, interleave):
            ag_outs, ag_ccs = ag_pair
            attnU = sbA.tile([128, DC, TL], bf, tag="attnU")
            k_srcs = [ag_outs[i].rearrange("r (p t) -> r p t", p=128)
                      for i in range(2)]
            v_srcs = [ag_outs[2 + i].rearrange(
                "r (kc p hl d) -> r p kc hl d", kc=KC // 2, p=128, hl=4)
                for i in range(2)]

            def fetch(pr):
                from concourse.bass import _add_dep_helper
                ktp_t = kthp.tile([128, T], f8, tag="kth")
                d = nc.sync.dma_start(ktp_t[:], k_srcs[pr % 2][pr // 2])
                _add_dep_helper(d.ins, ag_ccs[pr % 2].ins, sync=True,
                                reason="ktp waits AllGather K quarter")
                # V rows come 65 wide (64 dims + gathered ones column): AV
                # matmul row 64 then accumulates the softmax denominator.
                va = vahp.tile([128, KC, 2, DH + 1], f8, tag="vah")
                hl0 = (2 * pr) % 4
                for q in range(4):  # split across DMA queues + V halves
                    d = nc.sync.dma_start(
                        va[:, 4 * q:4 * q + 4, :, :],
                        v_srcs[q // 2][pr // 2, :, 4 * (q % 2):4 * (q % 2) + 4,
                                       hl0:hl0 + 2, :])
                    _add_dep_helper(d.ins, ag_ccs[2 + q // 2].ins, sync=True,
                                    reason="va waits AllGather V half")
                return ktp_t, va

            def av_emit(pend, kcs):
                pexp, pva, pav0, pav1, _ = pend
                for kc in kcs:
                    e_t, j = pexp[kc]
                    st, sp = kc == 0, kc == KC - 1
                    nc.tensor.matmul(pav0[0:DH + 1, :], pva[:, kc, 0, :],
                                     e_t[:, j, 0, :], start=st, stop=sp)
                    nc.tensor.matmul(pav1[0:DH + 1, :], pva[:, kc, 1, :],
                                     e_t[:, j, 1, :], start=st, stop=sp)

            def av_copyout(pend):
                """Cheap DVE-only drain of the AV psum banks: unnormalized
                attn + denominator rows out to SBUF.  Keeps the next pr's AV
                matmuls from head-of-line-blocking behind the normalization
                chain in the in-order PE queue."""
                _, _, pav0, pav1, ppr = pend
                den_sb = recp.tile([33, 512], fp32, tag="densb")
                nc.vector.tensor_copy(den_sb[0:1, :], pav0[DH:DH + 1, :])
                nc.vector.tensor_copy(den_sb[32:33, :], pav1[DH:DH + 1, :])
                nc.vector.tensor_copy(attnU[0:64, ppr, :], pav0[0:64, :])
                nc.vector.tensor_copy(attnU[64:128, ppr, :], pav1[0:64, :])
                if DEBUG_TAPS and attn_out is tap_attn_ref[0]:
                    nc.sync.dma_start(aps["tap_den"][ppr, 0], den_sb[0:1, :])
                    nc.sync.dma_start(aps["tap_den"][ppr, 1],
                                      den_sb[32:33, :])
                return (den_sb, ppr)

            def av_norm(nstate):
                den_sb, ppr = nstate
                rec = recp.tile([33, 512], fp32, tag="rec")
                nc.vector.reciprocal_approx_fast(out=rec[:, :],
                                                 in_=den_sb[0:33, :])
                plane = qkB_tile()[:, 0, 0, :]
                for hf in range(2):
                    nc.tensor.matmul(
                        plane[64 * hf:64 * hf + 64, :],
                        ones_f32[32 * hf:32 * hf + 1, :],
                        rec[32 * hf:32 * hf + 1, :],
                        start=True, stop=True,
                        tile_position=(32 * hf, 64 * hf))
                plane_sb = small2.tile([128, 512], bf, tag="plane_sb")
                nc.vector.tensor_copy(plane_sb[:], plane[:])
                nc.vector.tensor_mul(out=attn_out[:, ppr, :],
                                     in0=attnU[:, ppr, :],
                                     in1=plane_sb[:])

            # even prs first: they only need the K oc0 quarter, which lands
            # first -- keeps the pipeline full while oc1/V still gather
            PR_ORDER = [0, 2, 4, 6, 1, 3, 5, 7]
            pend = None
            norm_pending = None
            pref = fetch(PR_ORDER[0])
            for it in range(DC + 1):
                if pend is not None:
                    pav0 = ps_av.tile([128, 512], fp32, tag="av")
                    pav1 = ps_ds.tile([128, 512], fp32, tag="ds", name="pav1")
                    pend = (pend[0], pend[1], pav0, pav1, pend[4])
                    av_kc = 0
                if it == DC:
                    if pend is not None:
                        av_emit(pend, range(KC))
                        if norm_pending is not None:
                            av_norm(norm_pending)
                            norm_pending = None
                        av_norm(av_copyout(pend))
                    break
                pr = PR_ORDER[it]
                ktp_t, va = pref
                if it + 1 < DC:
                    pref = fetch(PR_ORDER[it + 1])

                exp_tiles = {}
                extra = list(interleave[it]) if interleave else []
                ei = 0
                for ci, (kind, kc0, nkc) in enumerate(KC_SCHED):
                    if ci == 2 and norm_pending is not None:
                        av_norm(norm_pending)
                        norm_pending = None
                    if pend is not None:
                        av_emit(pend, range(av_kc, av_kc + nkc))
                        av_kc += nkc
                    ps = qkA_tile() if kind == "A" else qkB_tile()
                    for j in range(nkc):
                        kc = kc0 + j
                        for hf in range(2):
                            po = 64 * hf
                            nc.tensor.matmul(
                                ps[:, j, hf, :],
                                ktp_t[po:po + 64, kc * 128:(kc + 1) * 128],
                                qt[po:po + 64, pr, 0:TL],
                                start=True, stop=True, tile_position=(po, 0))
                    if mask_sb is not None:
                        for j in range(nkc):
                            for hf in range(2):
                                nc.vector.tensor_add(
                                    out=ps[:, j, hf, :], in0=ps[:, j, hf, :],
                                    in1=mask_sb[:, kc0 + j, :])
                    pool = expA_p if kind == "A" else expB_p
                    e_t = pool.tile([128, nkc, 2, 512], bf, tag="exp" + kind)
                    nc.scalar.activation(
                        out=e_t[:].rearrange("p a b q -> p (a b q)"),
                        in_=ps[:, 0:nkc].rearrange("p a b q -> p (a b q)"),
                        func=AF.Exp, scale=SCALE)
                    for j in range(nkc):
                        exp_tiles[kc0 + j] = (e_t, j)
                if pend is not None:
                    norm_pending = av_copyout(pend)
                while ei < len(extra):
                    extra[ei]()
                    ei += 1
                pend = (exp_tiles, va, None, None, pr)

        # ================= emission =================
        def mark(name):
            _PHASE_MARKS.append((name, nc.next_id()))

        qt_sa = sbA.tile([128, DC, TL], bf, tag="qt")
        attn1 = sbA.tile([128, DC, TL], bf, tag="attn")

        with ExitStack() as src_ctx:
            srcp = src_ctx.enter_context(tc.tile_pool(name="srcp", bufs=8))

            # SA K/V first; each AG quarter kicked as soon as its staging
            # is emitted so the gather overlaps the rest of the prologue.
            mark("sa_kv")
            ag_sa = kv_and_ag("decT_f8", "w_sak", "w_sav", ag_sa_p)
            mark("sa_q")
            wq = weight_tile("w_saq")
            linear(wq, xloc_b, q_drain(qt_sa, "saq"))
            mark("ca_kv")
            ag_ca = kv_and_ag("encT_f8", "w_cak", "w_cav", ag_ca_p)
            mark("sa_attn")
            wo = weight_tile("w_sao")
            tap_attn_ref[0] = attn1
            attention(qt_sa, ag_sa, mask_sa, attn1, None)

        # SA O-proj + bias + residual -> x fp32 (+ squares for LN1)
        mark("sa_o")
        x_sb = sbX.tile([128, DC, TL], fp32, tag="xf")
        sq_sb = sbA.tile([128, DC, TL], bf, tag="sq")
        stat1 = ln_stat_tile()
        linear(wo, attn1, o_drain(x_sb, sq_sb, "sao", xloc_b, stat1))

        if DEBUG_TAPS:
            nc.sync.dma_start(
                aps["tap_attn1"].rearrange("(c p) t -> p c t", p=128), attn1[:])
            nc.sync.dma_start(
                aps["tap_x"].rearrange("(c p) t -> p c t", p=128), x_sb[:])
        mark("ln1")
        wqc = weight_tile("w_caq")
        x1_b = sbX.tile([128, DC, TL], bf, tag="xb")
        qk4 = [None]

        def ln1_qk(dcc):
            # ride CA-Q oc0-3 on the 4 qkA bank slices as x1 chunks appear
            if qk4[0] is None:
                qk4[0] = qkA_tile()
            for oc in range(4):
                nc.tensor.matmul(
                    qk4[0][:, oc // 2, oc % 2, :],
                    wqc[:, dcc, oc * 128:(oc + 1) * 128], x1_b[:, dcc, :],
                    start=(dcc == 0), stop=(dcc == DC - 1))

        qt_ca = sbA.tile([128, DC, TL], bf, tag="qt")
        layernorm(0, x_sb, sq_sb, x1_b, stat1, qk_cb=ln1_qk)
        qdr = q_drain(qt_ca, "caq")
        for oc in range(4):
            qdr(oc, qk4[0][:, oc // 2, oc % 2, :])
        for oc in range(4, DC):
            p = lin_psum(oc)
            for dcc in range(DC):
                nc.tensor.matmul(
                    p, wqc[:, dcc, oc * 128:(oc + 1) * 128], x1_b[:, dcc, :],
                    start=(dcc == 0), stop=(dcc == DC - 1))
            qdr(oc, p)
        if DEBUG_TAPS:
            nc.sync.dma_start(
                aps["tap_x1b"].rearrange("(c p) t -> p c t", p=128), x1_b[:])

        # CA
        mark("ca_q")
        mark("ca_attn")
        attn2 = sbA.tile([128, DC, TL], bf, tag="attn")
        woc = weight_tile("w_cao")
        attention(qt_ca, ag_ca, mask_ca, attn2, None)

        mark("ca_o")
        x2_sb = sbX.tile([128, DC, TL], fp32, tag="xf")
        sq2_sb = sbA.tile([128, DC, TL], bf, tag="sq")
        stat2 = ln_stat_tile()
        linear(woc, attn2, o_drain(x2_sb, sq2_sb, "cao", x1_b, stat2))

    mark("ln2")
    x2_b = sbX.tile([128, DC, TL], bf, tag="xb")
    layernorm(1, x2_sb, sq2_sb, x2_b, stat2)

    # ---------------- FFN ----------------
    with ExitStack() as ffn_ctx:
        mark("ffn1")
        ffnp = ffn_ctx.enter_context(tc.tile_pool(name="ffnp", bufs=1))
        w1p = ffn_ctx.enter_context(tc.tile_pool(name="w1p", bufs=4))
        w2p = ffn_ctx.enter_context(tc.tile_pool(name="w2p", bufs=2))
        ht = ffnp.tile([128, OC1, TL], bf, tag="ht")
        for oc in range(OC1):
            w1 = w1p.tile([128, DC, 128], bf, tag="w1")
            nc.sync.dma_start(
                w1[:],
                aps["w1t"].rearrange("(c p) o -> p c o",
                                     p=128)[:, :, oc * 128:(oc + 1) * 128])
            p = lin_psum(oc)
            for dcc in range(DC):
                nc.tensor.matmul(p, w1[:, dcc, :], x2_b[:, dcc, :],
                                 start=(dcc == 0), stop=(dcc == DC - 1))
            nc.scalar.activation(out=ht[:, oc, :], in_=p, func=AF.Relu,
                                 bias=bcol("b1", oc), scale=1.0)
        mark("ffn2")
        y_sb = sbX.tile([128, DC, TL], fp32, tag="xf")
        sq3_sb = sbA.tile([128, DC, TL], bf, tag="sq")
        stat3 = ln_stat_tile()
        dr3 = o_drain(y_sb, sq3_sb, "b2", x2_b, stat3)
        for oc in range(DC):
            w2 = w2p.tile([128, OC1, 128], bf, tag="w2")
            nc.sync.dma_start(
                w2[:],
                aps["w2t"].rearrange("(c p) o -> p c o",
                                     p=128)[:, :, oc * 128:(oc + 1) * 128])
            p = lin_psum(oc)
            for kc in range(OC1):
                nc.tensor.matmul(p, w2[:, kc, :], ht[:, kc, :],
                                 start=(kc == 0), stop=(kc == OC1 - 1))
            dr3(oc, p)

        mark("ln3")
        out_f = sbX.tile([128, DC, TL], fp32, tag="xf")
        outT = aps["outT"].rearrange("(c p) t -> p c t", p=128)
        layernorm(2, y_sb, sq3_sb, out_f, stat3,
                  out_cb=lambda dcc: nc.sync.dma_start(outT[:, dcc, :],
                                                       out_f[:, dcc, :]))
        mark("end")


def _build(use_mask):
    import concourse.bass as bass
    import concourse.tile as tile
    from concourse import bacc, mybir
    dt = mybir.dt
    nc = bacc.Bacc("TRN2", target_bir_lowering=False, debug=False,
                   num_devices=N_CORES)
    aps = {}

    def inp(name, shape, dtype):
        aps[name] = nc.dram_tensor(name, shape, dtype, kind="ExternalInput").ap()

    inp("decT_f8", [D, T], dt.float8e4)
    inp("encT_f8", [D, T], dt.float8e4)
    inp("xlocT_bf", [D, TL], dt.bfloat16)
    for nm in ("w_saq", "w_sao", "w_caq", "w_cao"):
        inp(nm, [D, D], dt.bfloat16)
    for nm in ("w_sak", "w_sav", "w_cak", "w_cav"):
        inp(nm, [D, KSLICE], dt.bfloat16)
    inp("w1t", [D, FFN], dt.bfloat16)
    inp("w2t", [FFN, D], dt.bfloat16)
    inp("bias_cols", [128, 72], dt.float32)
    inp("gbe_all", [128, 48], dt.float32)
    if use_mask:
        inp("amask_saT", [T, TL], dt.bfloat16)
        inp("amask_caT", [T, TL], dt.bfloat16)
    aps["outT"] = nc.dram_tensor("outT", [D, TL], dt.float32,
                                 kind="ExternalOutput").ap()
    if DEBUG_TAPS:
        aps["tap_attn1"] = nc.dram_tensor("tap_attn1", [D, TL], dt.bfloat16,
                                          kind="ExternalOutput").ap()
        aps["tap_x"] = nc.dram_tensor("tap_x", [D, TL], dt.float32,
                                      kind="ExternalOutput").ap()
        aps["tap_x1b"] = nc.dram_tensor("tap_x1b", [D, TL], dt.bfloat16,
                                        kind="ExternalOutput").ap()
        aps["tap_den"] = nc.dram_tensor("tap_den", [DC, 2, 512], dt.float32,
                                        kind="ExternalOutput").ap()
        aps["tap_rec"] = nc.dram_tensor("tap_rec", [DC, 2, 512], dt.float32,
                                        kind="ExternalOutput").ap()

    from contextlib import ExitStack
    with tile.TileContext(nc) as tc:
        with ExitStack() as ctx:
            _emit(ctx, tc, nc, aps, use_mask)
    nc.compile()
    return nc


def _make_runner(nc):
    import jax
    from jax.sharding import Mesh, PartitionSpec
    from jax.experimental.shard_map import shard_map
    from concourse import bass2jax, mybir

    bass2jax.install_neuronx_cc_hook()
    part_name = nc.partition_id_tensor.name if nc.partition_id_tensor else None
    in_names, out_names, out_avals = [], [], []
    for alloc in nc.m.functions[0].allocations:
        if not isinstance(alloc, mybir.MemoryLocationSet):
            continue
        name = alloc.memorylocations[0].name
        if alloc.kind == "ExternalInput":
            if name != part_name:
                in_names.append(name)
        elif alloc.kind == "ExternalOutput":
            out_names.append(name)
            out_avals.append(jax.core.ShapedArray(tuple(alloc.tensor_shape),
                                                  mybir.dt.np(alloc.dtype)))
    n_params = len(in_names)
    all_names = list(in_names + out_names)
    if part_name is not None:
        all_names.append(part_name)
    all_names = tuple(all_names)

    def _body(*args):
        operands = list(args)
        if part_name is not None:
            operands.append(bass2jax.partition_id_tensor())
        return tuple(bass2jax._bass_exec_p.bind(
            *operands, out_avals=tuple(out_avals), in_names=all_names,
            out_names=tuple(out_names), lowering_input_output_aliases=(),
            sim_require_finite=True, sim_require_nnan=True, nc=nc))

    devices = jax.devices()[:N_CORES]
    mesh = Mesh(np.asarray(devices), ("core",))
    spec = (PartitionSpec("core"),)
    nin = n_params + len(out_names)
    jfn = jax.jit(
        shard_map(_body, mesh=mesh, in_specs=spec * nin,
                  out_specs=spec * len(out_names), check_rep=False),
        donate_argnums=tuple(range(n_params, nin)), keep_unused=True)

    from jax.sharding import NamedSharding
    shard = NamedSharding(mesh, PartitionSpec("core"))

    def run(in_maps, timeit=False):
        concat_in = [np.concatenate([np.asarray(in_maps[c][n]) for c in range(N_CORES)],
                                    axis=0) for n in in_names]
        zeros = [np.zeros((N_CORES * a.shape[0],) + tuple(a.shape[1:]), a.dtype)
                 for a in out_avals]
        dev_in = [jax.device_put(a, shard) for a in concat_in]
        out = jfn(*dev_in, *[jax.device_put(z, shard) for z in zeros])
        jax.block_until_ready(out)
        times = []
        if timeit:
            # batched back-to-back dispatch; the k-slope of total wall time
            # isolates per-exec device time from fixed batch overhead.
            for _rep in range(2):
                totals = {}
                for k in (4, 44):
                    zsets = [[jax.device_put(z, shard) for z in zeros]
                             for _ in range(k)]
                    jax.block_until_ready(zsets)
                    t0 = time.perf_counter()
                    outs = [jfn(*dev_in, *zs) for zs in zsets]
                    jax.block_until_ready(outs)
                    totals[k] = time.perf_counter() - t0
                times.append((totals[44] - totals[4]) / 40.0)
        per_core = [{n: np.asarray(out[i]).reshape(N_CORES, *out_avals[i].shape)[c]
                     for i, n in enumerate(out_names)} for c in range(N_CORES)]
        return per_core, times

    return run


def _prep_inputs(dec, enc, t_mask, s_mask, weights):
    """Returns (in_maps, use_mask)."""
    use_mask_sa = not bool(np.all(t_mask != 0))
    use_mask_ca = not bool(np.all(s_mask != 0))
    use_mask = use_mask_sa or use_mask_ca

    kvT = weights["_kvT"]
    shared = {k: v for k, v in weights.items() if not k.startswith("_")}
    in_maps = []
    for c in range(N_CORES):
        b, s = c // CPB, c % CPB
        rows = slice(s * TL, (s + 1) * TL)
        osl = slice(s * KSLICE, (s + 1) * KSLICE)
        m = dict(shared)
        for nm in ("w_sak", "w_sav", "w_cak", "w_cav"):
            m[nm] = np.ascontiguousarray(kvT[nm][:, osl])
        decT = np.ascontiguousarray(dec[b].T)
        m["decT_f8"] = decT.astype(f8np)
        m["encT_f8"] = np.ascontiguousarray(enc[b].T).astype(f8np)
        xloc = np.ascontiguousarray(dec[b, rows].T)
        m["xlocT_bf"] = xloc.astype(bf16)
        if use_mask:
            am_sa = ((t_mask[0, 0] == 0) * MASK_NEG).astype(np.float32)
            am_ca = ((s_mask[0, 0] == 0) * MASK_NEG).astype(np.float32)
            m["amask_saT"] = np.ascontiguousarray(am_sa[rows].T).astype(bf16)
            m["amask_caT"] = np.ascontiguousarray(am_ca[rows].T).astype(bf16)
        in_maps.append(m)
    return in_maps, use_mask


def _pack_weights(kw):
    """Shared (core-independent) weight tensors."""
    w = {}
    for src, dst in (("sa_wq", "w_saq"), ("sa_wo", "w_sao"), ("ca_wq", "w_caq"),
                     ("ca_wo", "w_cao"), ("w1", "w1t"), ("w2", "w2t")):
        w[dst] = np.ascontiguousarray(np.asarray(kw[src]).T).astype(bf16)

    f32 = np.float32
    # O biases with the V bias folded in: bo' = bo + Wo @ bv
    b_sao = (np.asarray(kw["sa_bo"]) +
             np.asarray(kw["sa_wo"]) @ np.asarray(kw["sa_bv"])).astype(f32)
    b_cao = (np.asarray(kw["ca_bo"]) +
             np.asarray(kw["ca_wo"]) @ np.asarray(kw["ca_bv"])).astype(f32)
    cols = [np.asarray(kw["sa_bq"], f32).reshape(DC, 128).T,
            b_sao.reshape(DC, 128).T,
            np.asarray(kw["ca_bq"], f32).reshape(DC, 128).T,
            b_cao.reshape(DC, 128).T,
            np.asarray(kw["b2"], f32).reshape(DC, 128).T,
            np.asarray(kw["b1"], f32).reshape(OC1, 128).T]
    w["bias_cols"] = np.ascontiguousarray(np.concatenate(cols, axis=1))
    gbe = [np.asarray(kw[k]).astype(f32).reshape(DC, 128).T for k in
           ("g1", "be1", "g2", "be2", "g3", "be3")]
    w["gbe_all"] = np.ascontiguousarray(np.concatenate(gbe, axis=1))
    w["g_rows"] = np.concatenate(
        [np.asarray(kw[k], f32).reshape(1, -1) for k in ("g1", "g2", "g3")],
        axis=1).astype(bf16)
    # K/V weights are head-sharded per core: slices added in _prep_inputs.
    w["_kvT"] = {nm: np.ascontiguousarray(np.asarray(kw[src]).T).astype(bf16)
                 for src, nm in (("sa_wk", "w_sak"), ("sa_wv", "w_sav"),
                                 ("ca_wk", "w_cak"), ("ca_wv", "w_cav"))}
    return w


def _get_runner(use_mask):
    key = bool(use_mask)
    if key not in _CACHE:
        nc = _build(key)
        _CACHE[key] = _make_runner(nc)
    return _CACHE[key]


def kernel(dec, enc, t_mask, s_mask, timeit=False, **kw):
    dec = np.asarray(dec, dtype=np.float32)
    enc = np.asarray(enc, dtype=np.float32)
    weights = _pack_weights(kw)
    in_maps, use_mask = _prep_inputs(dec, enc, np.asarray(t_mask),
                                     np.asarray(s_mask), weights)
    run = _get_runner(use_mask)
    per_core, times = run(in_maps, timeit=timeit)
    out = np.empty((B, T, D), np.float32)
    for c in range(N_CORES):
        b, s = c // CPB, c % CPB
        out[b, s * TL:(s + 1) * TL] = per_core[c]["outT"].T
    if timeit:
        kernel._last_times = times
    return out

